# revision 27
# baseline (speedup 1.0000x reference)
"""Trainium2 Bass kernel for nn_MaxAttSentence.

Problem (per batch row b):
  1. segment sums: sums[s] = sum(attention[b, start_s:end_s]) for 128 sentences
  2. best = first argmax(sums); valid = max > 0; (bstart, bend) = startends[best]
     (or (0, 0) if not valid)
  3. output[b, m, :] = context[b, bstart + m, :] if m < bend - bstart else 0,
     for m in [0, 64)

Strategy: pure data parallel across 8 NeuronCores (4 batch rows per core).
On-device per core:
  - cumulative sums of attention via DVE scan over a [128 part, 128 free]
    chunked layout (partition p = 32*r + chunk, 128 elements per chunk) plus
    per-row chunk-offset prefix via a small strict-lower-triangular matmul.
  - csum gather at the 2*128 sentence boundary positions per row via one-hot
    matmuls on PE (chunk select) + per-partition one-hot reduce on DVE
    (position-in-chunk select).
  - argmax with first-max tie-break via max/equal/iota/min reduction.
  - data-dependent gather of 64 context rows per batch row via a
    register-offset DMA (values_load + bass.ds dynamic slice).
  - mask rows >= blen by a per-partition multiply; store.

The kernel reads only ~1.7 MB per core from HBM (attention, gathered context
rows, output) instead of the full 50 MB context shard.
"""

import numpy as np
from contextlib import ExitStack

import concourse.bass as bass
import concourse.bacc as bacc
import concourse.mybir as mybir
import concourse.tile as tile
from concourse import bass_utils

F32 = mybir.dt.float32
I32 = mybir.dt.int32
Alu = mybir.AluOpType
AX = mybir.AxisListType

B, L, D, S = 32, 4096, 768, 128
M = 64                 # MAX_SENT_LEN
NCORES = 8
R = B // NCORES        # 4 batch rows per core
CHUNK = 128            # scan chunk length (free dim of att layout)
NCH = L // CHUNK       # 32 chunks per batch row
PAD = M                # zero-pad rows appended to the context shard
BIG = 1.0e9


def _kernel_body(tc, att, se_i, se_r, ctx, out):
    nc = tc.nc
    with ExitStack() as ex:
        cp = ex.enter_context(tc.tile_pool(name="consts", bufs=1))
        wp = ex.enter_context(tc.tile_pool(name="work", bufs=1))
        pp = ex.enter_context(tc.tile_pool(name="ps", bufs=1, space="PSUM"))
        gp = ex.enter_context(tc.tile_pool(name="gps", bufs=4, space="PSUM"))
        sp = ex.enter_context(tc.tile_pool(name="scratch", bufs=2))
        op_ = ex.enter_context(tc.tile_pool(name="outp", bufs=4))

        # ---- constants (generated on device) ----
        # NOTE: every tensor read by a PE matmul must be produced on DVE
        # (nc.vector) — matmuls can carry only ONE semaphore wait, so all
        # their producers must sit behind a single engine's semaphore.
        it1 = cp.tile([128, 128], I32)
        nc.gpsimd.iota(it1, pattern=[[1, 128]], base=0, channel_multiplier=-1)
        ident = cp.tile([128, 128], F32)          # identity: 1.0 where f == p
        nc.vector.tensor_scalar(ident, it1, 0, None, Alu.is_equal)

        mstrict = cp.tile([128, 128], F32)        # strict lower: 1.0 where p < f
        nc.vector.tensor_scalar(mstrict, it1, 0, None, Alu.is_gt)

        colp_i = cp.tile([128, 1024], I32)        # value = partition index
        nc.gpsimd.iota(colp_i, pattern=[[0, 1024]], base=0, channel_multiplier=1)
        colp = cp.tile([128, 1024], F32)
        nc.vector.tensor_copy(colp, colp_i)

        fj_i = cp.tile([128, 128], I32)           # value = free index
        nc.gpsimd.iota(fj_i, pattern=[[1, 128]], base=0, channel_multiplier=0)
        fj = cp.tile([128, 128], F32)
        nc.vector.tensor_copy(fj, fj_i)

        ones1 = cp.tile([1, 128], F32)
        nc.vector.memset(ones1, 1.0)

        ir_i = cp.tile([4, 128], I32)             # value = free index (4 rows)
        nc.gpsimd.iota(ir_i, pattern=[[1, 128]], base=0, channel_multiplier=0)
        irf = cp.tile([4, 128], F32)
        nc.vector.tensor_copy(irf, ir_i)

        # rowoff[i, c] = 32 * (c >> 1) for c = 2r + side
        rowoff = cp.tile([128, 8], I32)
        nc.gpsimd.iota(rowoff, pattern=[[16, 8]], base=0, channel_multiplier=0)
        nc.vector.tensor_scalar(rowoff, rowoff, -32, None, Alu.bitwise_and)

        ro4_i = cp.tile([4, 1], I32)              # value = 4096 * partition
        nc.gpsimd.iota(ro4_i, pattern=[[0, 1]], base=0, channel_multiplier=L)
        ro4f = cp.tile([4, 1], F32)
        nc.vector.tensor_copy(ro4f, ro4_i)

        # ---- input loads ----
        att_sb = wp.tile([128, 128], F32)
        nc.sync.dma_start(att_sb, att)
        sei_sb = wp.tile([128, 8], I32)
        nc.sync.dma_start(sei_sb, se_i)
        ser_sb = wp.tile([4, 256], I32)
        nc.sync.dma_start(ser_sb, se_r)

        # ---- prefix sums ----
        # opall: col 0 = global exclusive chunk offset (prefix across ALL 128
        # chunks; cross-row contributions cancel in the end-start difference),
        # cols 1..128 = inclusive within-chunk scan.
        opall = wp.tile([128, 129], F32)
        nc.vector.tensor_tensor_scan(
            opall[:, 1:129], att_sb, att_sb, 0.0, Alu.add, Alu.bypass
        )
        offs_ps = pp.tile([128, 1], F32, tag="pp")
        nc.tensor.matmul(offs_ps, mstrict, opall[:, 128:129], start=True, stop=True)
        nc.vector.tensor_copy(opall[:, 0:1], offs_ps)

        # ---- boundary index math (p in [0, 4096] -> chunk + in-chunk pos) ----
        # V(p) = global inclusive csum at position r*4096 + p - 1
        #      = offs[c2g] + partial[c2g, (p-1) & 127],  c2g = 32r + ((p-1)>>7)
        # p == 0 in row 0 gives c2g = -1 (matches no partition -> V = 0, the
        # correct global csum before position 0).
        secl = wp.tile([128, 8], I32)
        nc.vector.tensor_scalar(secl, sei_sb, L, 0, Alu.min, Alu.max)
        q_t = wp.tile([128, 8], I32)
        nc.vector.tensor_scalar(q_t, secl, -1, None, Alu.add)
        c2_t = wp.tile([128, 8], I32)
        nc.vector.tensor_scalar(c2_t, q_t, 7, None, Alu.arith_shift_right)
        f_t = wp.tile([128, 8], I32)
        nc.vector.tensor_scalar(f_t, q_t, 127, None, Alu.bitwise_and)
        c2g_t = wp.tile([128, 8], I32)
        nc.vector.tensor_tensor(c2g_t, c2_t, rowoff, Alu.add)
        c2g_f = wp.tile([128, 8], F32)
        nc.vector.tensor_copy(c2g_f, c2g_t)
        f_f = wp.tile([128, 8], F32)
        nc.vector.tensor_copy(f_f, f_t)

        # ---- one-hot chunk select (PE) ----
        # flatten c2g [128 i, 8 c] -> [1, 1024] (flat pos = 128*c + i) via a
        # small SBUF->SBUF DMA, then broadcast along partitions with K=1
        # outer-product matmuls.
        # flat position of c2g[i, c] is 8*i + c (i-major: the DMA's final dim
        # must be contiguous on both sides)
        c2g_flat = wp.tile([1, 1024], F32)
        nc.sync.dma_start(
            c2g_flat[:, :].rearrange("p (i c) -> p i c", c=8),
            c2g_f[:, :],
        )
        # funnel the DMA through DVE so the bc matmuls have a single-engine
        # dependency (single-wait constraint)
        c2g_fsb = wp.tile([1, 1024], F32)
        nc.vector.tensor_copy(c2g_fsb, c2g_flat)
        bc_ps = pp.tile([128, 1024], F32, tag="pp")
        nc.tensor.matmul(bc_ps[:, 0:512], ones1, c2g_fsb[0:1, 0:512],
                         start=True, stop=True)
        nc.tensor.matmul(bc_ps[:, 512:1024], ones1, c2g_fsb[0:1, 512:1024],
                         start=True, stop=True)
        # write c-major (slot c at columns [128c, 128c+128)) so each G-matmul
        # reads a contiguous stationary slice
        onehots = wp.tile([128, 1024], F32)
        nc.vector.tensor_tensor(
            onehots[:, :].rearrange("p (c i) -> p i c", c=8), bc_ps, colp,
            Alu.is_equal)

        # ---- gather csum values: per (row, side) ----
        # (tensor_tensor_reduce is a custom DVE op that faults on this runtime;
        # use plain mult + reduce + add instead)
        gsel = wp.tile([128, 8], F32)
        for c in range(2 * R):
            g_ps = gp.tile([128, 129], F32, tag="g")
            nc.tensor.matmul(
                g_ps, onehots[:, 128 * c:128 * (c + 1)], opall[:, 0:129],
                start=True, stop=True,
            )
            joh = sp.tile([128, 128], F32, tag="joh")
            nc.vector.tensor_scalar(joh, fj, f_f[:, c:c + 1], None, Alu.is_equal)
            prod = sp.tile([128, 128], F32, tag="prod")
            nc.vector.tensor_tensor(prod, joh, g_ps[:, 1:129], Alu.mult)
            red = sp.tile([128, 1], F32, tag="red")
            nc.vector.tensor_reduce(red, prod, AX.X, Alu.add)
            nc.vector.tensor_tensor(gsel[:, c:c + 1], red, g_ps[:, 0:1], Alu.add)

        # ---- segment sums + first-max argmax ----
        sums = wp.tile([128, 4], F32)
        nc.vector.tensor_tensor(sums, gsel[:, 1:8:2], gsel[:, 0:8:2], Alu.subtract)
        sumsT_ps = pp.tile([4, 128], F32, tag="pp")
        nc.tensor.matmul(sumsT_ps, sums, ident, start=True, stop=True)
        sumsT = wp.tile([4, 128], F32)
        nc.vector.tensor_copy(sumsT, sumsT_ps)
        rowmax = wp.tile([4, 1], F32)
        nc.vector.tensor_reduce(rowmax, sumsT, AX.X, Alu.max)
        eqm = wp.tile([4, 128], F32)
        nc.vector.tensor_scalar(eqm, sumsT, rowmax[:, 0:1], None, Alu.is_equal)
        cand = wp.tile([4, 128], F32)
        nc.vector.tensor_scalar(cand, eqm, -BIG, BIG, Alu.mult, Alu.add)
        nc.vector.tensor_tensor(cand, cand, irf, Alu.add)
        bidx = wp.tile([4, 1], F32)
        nc.vector.tensor_reduce(bidx, cand, AX.X, Alu.min)
        ohb = wp.tile([4, 128], F32)
        nc.vector.tensor_scalar(ohb, irf, bidx[:, 0:1], None, Alu.is_equal)

        startsT_f = wp.tile([4, 128], F32)
        nc.vector.tensor_copy(startsT_f, ser_sb[:, 0:256:2])
        endsT_f = wp.tile([4, 128], F32)
        nc.vector.tensor_copy(endsT_f, ser_sb[:, 1:256:2])
        prods = sp.tile([4, 128], F32, tag="prods")
        bstart = wp.tile([4, 1], F32)
        nc.vector.tensor_tensor(prods, ohb, startsT_f, Alu.mult)
        nc.vector.tensor_reduce(bstart, prods, AX.X, Alu.add)
        prode = sp.tile([4, 128], F32, tag="prode")
        bend = wp.tile([4, 1], F32)
        nc.vector.tensor_tensor(prode, ohb, endsT_f, Alu.mult)
        nc.vector.tensor_reduce(bend, prode, AX.X, Alu.add)
        valid = wp.tile([4, 1], F32)
        nc.vector.tensor_scalar(valid, rowmax, 0.0, None, Alu.is_gt)
        blen = wp.tile([4, 1], F32)
        nc.vector.tensor_tensor(blen, bend, bstart, Alu.subtract)
        nc.vector.tensor_tensor(blen, blen, valid, Alu.mult)
        bstart_v = wp.tile([4, 1], F32)
        nc.vector.tensor_tensor(bstart_v, bstart, valid, Alu.mult)
        nc.vector.tensor_scalar(bstart_v, bstart_v, float(L), 0.0, Alu.min, Alu.max)
        rowstart_f = wp.tile([4, 1], F32)
        nc.vector.tensor_tensor(rowstart_f, bstart_v, ro4f, Alu.add)
        rowstart_i = wp.tile([4, 1], I32)
        nc.vector.tensor_copy(rowstart_i, rowstart_f)

        # ---- row mask (1.0 where m < blen) transposed to [64, 4] ----
        maskT = wp.tile([4, 64], F32)
        nc.vector.tensor_scalar(maskT, irf[:, 0:64], blen[:, 0:1], None, Alu.is_lt)
        maskTT_ps = pp.tile([64, 4], F32, tag="pp")
        nc.tensor.matmul(maskTT_ps, maskT, ident[0:4, 0:4], start=True, stop=True)
        maskTT = wp.tile([64, 4], F32)
        nc.vector.tensor_copy(maskTT, maskTT_ps)

        # ---- dynamic gather of 64 context rows per batch row + mask + store ----
        for r in range(R):
            # the runtime bounds assert uses a halt instruction that is not
            # supported under the axon/PJRT execution path; bstart is clamped
            # to [0, L] on device so the static bounds are guaranteed.
            _, (rv,) = nc.values_load_multi_w_load_instructions(
                rowstart_i[r:r + 1, 0:1],
                engines=(mybir.EngineType.SP,),
                min_val=r * L,
                max_val=(r + 1) * L,
                skip_runtime_bounds_check=True,
            )
            gt = op_.tile([M, D], F32, tag="gt")
            nc.sync.dma_start(gt, ctx[bass.ds(rv, M), :])
            ot = op_.tile([M, D], F32, tag="ot")
            if r < 2:
                nc.vector.tensor_scalar(
                    ot, gt, maskTT[:, r:r + 1], None, Alu.mult
                )
            else:
                nc.scalar.activation(
                    ot, gt, mybir.ActivationFunctionType.Copy,
                    scale=maskTT[:, r:r + 1],
                )
            nc.sync.dma_start(out[M * r:M * (r + 1), :], ot)


def build_bass():
    nc = bacc.Bacc("TRN2", target_bir_lowering=False, debug=False,
                   num_devices=NCORES)
    att = nc.dram_tensor("att", [R * NCH, CHUNK], F32, kind="ExternalInput")
    se_i = nc.dram_tensor("se_i", [S, 2 * R], I32, kind="ExternalInput")
    se_r = nc.dram_tensor("se_r", [R, 2 * S], I32, kind="ExternalInput")
    ctx = nc.dram_tensor("ctx", [R * L + PAD, D], F32, kind="ExternalInput")
    out = nc.dram_tensor("out", [R * M, D], F32, kind="ExternalOutput")
    with tile.TileContext(nc) as tc:
        _kernel_body(tc, att.ap(), se_i.ap(), se_r.ap(), ctx.ap(), out.ap())
    nc.compile()
    return nc


def make_in_maps(startends, attention, context):
    in_maps = []
    for core in range(NCORES):
        sl = slice(core * R, (core + 1) * R)
        se_l = np.ascontiguousarray(startends[sl]).astype(np.int32)
        att_l = np.ascontiguousarray(attention[sl], dtype=np.float32)
        ctx_l = np.asarray(context[sl], dtype=np.float32).reshape(R * L, D)
        ctx_p = np.empty((R * L + PAD, D), dtype=np.float32)
        ctx_p[:R * L] = ctx_l
        ctx_p[R * L:] = 0.0
        in_maps.append({
            "att": att_l.reshape(R * NCH, CHUNK),
            "se_i": np.ascontiguousarray(se_l.transpose(1, 0, 2)).reshape(S, 2 * R),
            "se_r": se_l.reshape(R, 2 * S),
            "ctx": ctx_p,
        })
    return in_maps


_NC_CACHE = []


def kernel(startends, attention, context):
    if not _NC_CACHE:
        _NC_CACHE.append(build_bass())
    nc = _NC_CACHE[0]
    in_maps = make_in_maps(startends, attention, context)
    res = bass_utils.run_bass_kernel_spmd(nc, in_maps, core_ids=list(range(NCORES)))
    outs = [res.results[c]["out"].reshape(R, M, D) for c in range(NCORES)]
    return np.concatenate(outs, axis=0)
